# revision 1
# baseline (speedup 1.0000x reference)
"""Contrastive-loss kernel for Trainium2, 8 NeuronCores — feature-Gram method.

Math
----
reference:
    yn  = ys / clip(||ys||, 1e-6)          (row-normalize)
    cos = yn @ yn.T                         [B, B]
    pair_loss = same ? relu(2 - cos)^2 : cos^2
    loss = sum(strict_lower(pair_loss)) / (B*(B-1)/2)

Since margin M = 2 and cos <= 1: relu(2-cos)^2 = (2-cos)^2 = cos^2 + 4*(1-cos),
so summing over the strict lower triangle (diagonal terms: cos_ii = 1,
pair_loss_ii = 1):
    sum_{i>j} pair_loss = (F1 - B)/2 + 2*(N2 - SS)
with
    F1 = sum_ij cos_ij^2 = ||Yn Yn^T||_F^2 = ||Yn^T Yn||_F^2   (!!)
    N2 = sum_c n_c^2                  (n_c = count of label c)
    SS = sum_c ||S_c||^2              (S_c = sum of yn rows with label c)

The Frobenius identity moves the O(B^2 D) sample-Gram to the D x D feature
Gram M = Yn^T Yn (same MAC count since D = B/2 here, but):
  * the contraction axis is the ROW axis — normalized rows feed the PE in
    natural [row-on-partition] layout, NO transpose phase;
  * the AllGather payload is fp8 rows: 8.4 MB total vs 16.8 MB bf16;
  * the epilogue is a plain square-reduce of M — no per-tile label
    masking (the baseline burned DVE on 16.7M is_equal/mult/reduce elems);
  * fp8 DoubleRow matmul (K=256/instruction) for ~2x PE throughput. The
    matmul terms carry ~0.2% of the loss value (the label term N2
    dominates and is exact-integer), so fp8 error is irrelevant at
    rel tol 2e-2.

SPMD note: one program runs on all 8 cores, so per-core variation lives
in input DATA only. Core c produces M[256c:256(c+1), :]. Its lhsT
(all 4096 rows x its 256 columns) cannot be a core-dependent slice of
the gathered buffer; instead the host supplies raw ys[:, 256c:256(c+1)]
(fp8) and the device scales rows by the gathered 32/||row|| factors,
which ride along the row AllGather as an extra fp8 column.

Device plan (SPMD, 8 cores):
 1. DMA own 512 rows (tile 0 split across both rings, squared on
    ACT+DVE in parallel — it gates the first gather chunk) + own raw
    column-slice xcols [4096, 256] fp8 on the gpsimd ring.
 2. Normalize own rows scaled by 32 into fp8e4 yn_own (+r32 column).
 3. Per-class sums S (own rows): one-hot fp8 matmul -> [10, 2048] +
    class counts via ones column; packed [10, 2049] f32.
 4. Collectives (gpsimd blocks on each): AllGather chunk a (own row-tile
    0, primes the pipeline), AllGather chunk b (tiles 1-3), then the S
    AllReduce (hidden under the matmuls).
 5. lhsT build: xcols * r32 strips -> fp8 [128, 32, 256] (DVE, hidden).
 6. Stream gathered rows to SBUF [128, 32, 2048] per rank-shard; K-outer
    DoubleRow matmuls (kc-plane pairs, stride-4 within chunk a) into two
    4-bank PSUM tiles [128, 2048] held across the accumulation.
 7. Epilogue: one wide ACT Square+accum (big0) in parallel with a DVE
    copy/mult/reduce (big1) -> F1 partials; square the AllReduced
    S/counts -> SS, n_c^2 partials.
 8. DMA out [128, 4] f32 partials; host reduces in float64 and rescales.
"""

import os
import sys

for _p in ("/opt/trn_rl_repo", "/root/.axon_site/_ro/trn_rl_repo"):
    if _p not in sys.path and os.path.isdir(_p):
        sys.path.append(_p)

import numpy as np

import concourse.bass as bass
import concourse.mybir as mybir
import concourse.tile as tile
from concourse.bass import ds, ts  # noqa: F401

F32 = mybir.dt.float32
BF16 = mybir.dt.bfloat16
FP8 = mybir.dt.float8e4
AF = mybir.ActivationFunctionType
ALU = mybir.AluOpType
DR = mybir.MatmulPerfMode.DoubleRow

P = 128       # partitions
NCLS = 10     # label classes
SCALE = 32.0  # fp8 pre-scale on yn


def _split_multi_waits(nc):
    """Split instructions carrying >1 semaphore wait.

    The walrus in this environment rejects compute instructions with more
    than one sync-wait command ("Too many sync wait commands"). Move the
    extra waits onto standalone EventSemaphore instructions inserted just
    before, on the same engine — semantically identical (the engine's
    sequencer blocks on each in order).
    """
    n_split = 0
    for fn in nc.m.functions:
        for bb in fn.blocks:
            new_insts = []
            for ins in bb.instructions:
                si = ins.sync_info
                if (
                    si is not None
                    and len(si.on_wait) > 1
                    and not isinstance(ins, mybir.InstEventSemaphore)
                ):
                    extra = list(si.on_wait[1:])
                    ins.sync_info = mybir.SyncInfo(
                        on_wait=[si.on_wait[0]], on_update=list(si.on_update)
                    )
                    for w in extra:
                        n_split += 1
                        ev = mybir.InstEventSemaphore(
                            name=f"antsplitwait_{n_split}_{ins.name}",
                            engine=ins.engine,
                            ins=[],
                            outs=[],
                            sync_info=mybir.SyncInfo(on_wait=[w], on_update=[]),
                            bass_nofuse=True,
                        )
                        new_insts.append(ev)
                new_insts.append(ins)
            bb.instructions = new_insts
    return n_split


def build_gram_loss(B=4096, D=2048, C=8, S=4):
    """Build the SPMD bass program (one nc, run on C cores). S unused."""
    assert B == 4096 and D == 2048 and C == 8
    Bs = B // C          # 512 rows per core
    RT = Bs // P         # 4 own row-tiles
    KC = B // P          # 32 gathered K chunks
    DC = D // C          # 256 own d1 columns

    nc = bass.Bass(num_devices=C)

    ys_mine = nc.dram_tensor("ys_mine", [Bs, D], F32, kind="ExternalInput")
    xcols = nc.dram_tensor("xcols", [P, (B // P) * DC], FP8, kind="ExternalInput")
    oh_mine = nc.dram_tensor("oh_mine", [P, RT * NCLS], F32, kind="ExternalInput")
    out_parts = nc.dram_tensor("out_parts", [P, 4], F32, kind="ExternalOutput")

    # +1 col: r32 = 32/||row|| rides along the row AllGather as fp8.
    # The gather is split in half-row chunks so it starts while phase A
    # finishes and streaming overlaps the second half.
    # Uneven split: chunk a = own row-tile 0 (ready first, primes the
    # pipeline), chunk b = row-tiles 1-3.
    cc_in = nc.dram_tensor("cc_in", [Bs, D + 1], FP8)
    cc_out_a = nc.dram_tensor("cc_out_a", [C * P, D + 1], FP8, addr_space="Shared")
    cc_out_b = nc.dram_tensor(
        "cc_out_b", [C * 3 * P, D + 1], FP8, addr_space="Shared"
    )
    cc_s_in = nc.dram_tensor("cc_s_in", [NCLS, D + 1], F32)
    cc_s_out = nc.dram_tensor("cc_s_out", [NCLS, D + 1], F32, addr_space="Shared")

    with tile.TileContext(nc) as tc:
        with (
            tc.tile_pool(name="const", bufs=1) as const_pool,
            tc.tile_pool(name="big", bufs=1) as big_pool,
            tc.tile_pool(name="ysin", bufs=1) as ys_pool,
            tc.tile_pool(name="sqscr", bufs=1) as sq_scratch_pool,
            tc.tile_pool(name="small", bufs=4) as small_pool,
            tc.tile_pool(name="acc", bufs=1) as acc_pool,
            tc.tile_pool(name="ep", bufs=2) as ep_pool,
            tc.tile_pool(name="red", bufs=4) as red_pool,
            tc.tile_pool(name="mm", bufs=1, space="PSUM") as mm_psum,
        ):
            # ---------------- constants / inputs ----------------
            eps_tile = const_pool.tile([P, 1], F32)
            nc.gpsimd.memset(eps_tile[:], 1e-6)
            c32_tile = const_pool.tile([P, 1], F32)
            nc.gpsimd.memset(c32_tile[:], SCALE)
            ones_f8 = const_pool.tile([P, 1], FP8)
            nc.gpsimd.memset(ones_f8[:], 1.0)

            oh_f32 = const_pool.tile([P, RT * NCLS], F32)
            nc.gpsimd.dma_start(out=oh_f32[:], in_=oh_mine[:, :])
            oh_f8 = const_pool.tile([P, RT * NCLS], FP8)
            nc.vector.tensor_copy(oh_f8[:], oh_f32[:])

            # own raw column-slice, all rows (for lhsT) — gpsimd ring is
            # otherwise idle before the collectives
            # host pre-packs to [p, kc, d] so this is one contiguous
            # transfer (SWDGE descriptor emission on the Q7 must not delay
            # the cc0 DMA / gather doorbell queued behind it)
            xc_sb = big_pool.tile([P, KC, DC], FP8)
            nc.gpsimd.dma_start(
                out=xc_sb[:, :, :],
                in_=xcols[:, :].rearrange("p (kc d) -> p kc d", kc=KC),
            )

            acc_f1 = acc_pool.tile([P, 1], F32)
            nc.vector.memset(acc_f1[:], 0.0)

            # ---------------- phase A: load + normalize own rows ----------
            # Tile 0 gates the first AllGather chunk: fetch its column
            # halves on both rings and square them on ACT + DVE in parallel.
            yn_own = big_pool.tile([P, RT, D + 1], FP8)
            r32_own = const_pool.tile([P, RT], F32)
            HD = D // 2
            ring = {0: nc.sync, 1: nc.scalar, 2: nc.sync, 3: nc.scalar}
            ys_tiles = []
            for t in range(RT):
                ys_t = ys_pool.tile([P, D], F32, name=f"ys_t{t}")
                ys_tiles.append(ys_t)
                if t == 0:
                    nc.sync.dma_start(
                        out=ys_t[:, 0:HD], in_=ys_mine[ts(t, P), 0:HD]
                    )
                    nc.scalar.dma_start(
                        out=ys_t[:, HD:D], in_=ys_mine[ts(t, P), HD:D]
                    )
                else:
                    ring[t].dma_start(out=ys_t[:], in_=ys_mine[ts(t, P), :])
            for t in range(RT):
                ys_t = ys_tiles[t]
                ssq = small_pool.tile([P, 1], F32)
                sq_scratch = sq_scratch_pool.tile([P, D], BF16)
                if t == 0:
                    ssq_r = small_pool.tile([P, 1], F32)
                    nc.scalar.activation(
                        sq_scratch[:, 0:HD], ys_t[:, 0:HD], AF.Square,
                        accum_out=ssq[:],
                    )
                    nc.vector.tensor_tensor(
                        sq_scratch[:, HD:D], ys_t[:, HD:D], ys_t[:, HD:D],
                        ALU.mult,
                    )
                    nc.vector.tensor_reduce(
                        ssq_r[:], sq_scratch[:, HD:D], mybir.AxisListType.X,
                        ALU.add,
                    )
                    nc.vector.tensor_tensor(ssq[:], ssq[:], ssq_r[:], ALU.add)
                else:
                    nc.scalar.activation(
                        sq_scratch[:], ys_t[:], AF.Square, accum_out=ssq[:]
                    )
                norm_t = small_pool.tile([P, 1], F32)
                nc.scalar.sqrt(norm_t[:], ssq[:])
                normc = small_pool.tile([P, 1], F32)
                nc.vector.tensor_tensor(normc[:], norm_t[:], eps_tile[:], ALU.max)
                r_t = small_pool.tile([P, 1], F32)
                nc.vector.reciprocal(r_t[:], normc[:])
                nc.vector.tensor_tensor(
                    r32_own[:, t : t + 1], r_t[:], c32_tile[:], ALU.mult
                )
                nc.vector.tensor_scalar_mul(
                    yn_own[:, t, 0:D], ys_t[:], r32_own[:, t : t + 1]
                )
                nc.scalar.copy(yn_own[:, t, D : D + 1], r32_own[:, t : t + 1])

                dmae2 = nc.gpsimd if t == 0 else ring[t]
                dmae2.dma_start(out=cc_in[ts(t, P), :], in_=yn_own[:, t, :])

            # ---------------- per-class sums S + counts (own rows) --------
            # Borrows main-loop PSUM tags (mm0/mm1) — runs before the Gram
            # matmuls; the tile framework serializes on the WAR deps.
            s_own = const_pool.tile([NCLS, D + 1], F32)
            for cb in range(4):
                ps_s = mm_psum.tile([P, D], F32, tag="big0")
                for t in range(RT):
                    nc.tensor.matmul(
                        ps_s[0:NCLS, ts(cb, 512)],
                        lhsT=oh_f8[:, ts(t, NCLS)],
                        rhs=yn_own[:, t, ts(cb, 512)],
                        start=(t == 0),
                        stop=(t == RT - 1),
                    )
                nc.scalar.copy(s_own[:, ts(cb, 512)], ps_s[0:NCLS, ts(cb, 512)])
            ps_c = mm_psum.tile([P, D], F32, tag="big1")
            for t in range(RT):
                nc.tensor.matmul(
                    ps_c[0:NCLS, 0:1],
                    lhsT=oh_f8[:, ts(t, NCLS)],
                    rhs=ones_f8[:],
                    start=(t == 0),
                    stop=(t == RT - 1),
                )
            nc.scalar.copy(s_own[:, D : D + 1], ps_c[0:NCLS, 0:1])

            # ---------------- collectives ----------------
            # gpsimd blocks on each collective's completion, so the two
            # row-gathers go first; the S DMA + AllReduce trail behind.
            nc.gpsimd.collective_compute(
                "AllGather",
                ALU.bypass,
                replica_groups=[list(range(C))],
                ins=[cc_in[0:P, :]],
                outs=[cc_out_a[:, :]],
            )
            nc.gpsimd.collective_compute(
                "AllGather",
                ALU.bypass,
                replica_groups=[list(range(C))],
                ins=[cc_in[P:Bs, :]],
                outs=[cc_out_b[:, :]],
            )
            nc.gpsimd.dma_start(out=cc_s_in[:, :], in_=s_own[:])
            nc.gpsimd.collective_compute(
                "AllReduce",
                ALU.add,
                replica_groups=[list(range(C))],
                ins=[cc_s_in[:, :]],
                outs=[cc_s_out[:, :]],
            )

            # per chunk: norm strips first (tiny, unblock the lhsT build),
            # then the row shards. Rank rc's tile t lands at kc 4*rc+t.
            gath = big_pool.tile([P, KC, D], FP8)
            r32s = big_pool.tile([P, KC, 1], FP8)
            r32f = big_pool.tile([P, KC, 1], F32)
            for rc in range(C):
                dmae2 = nc.scalar if rc % 2 == 0 else nc.sync
                dmae2.dma_start(
                    out=r32s[:, 4 * rc : 4 * rc + 1, :],
                    in_=cc_out_a[rc * P : (rc + 1) * P, D : D + 1].rearrange(
                        "(kc p) d -> p kc d", p=P
                    ),
                )
                nc.scalar.copy(
                    r32f[:, 4 * rc : 4 * rc + 1, :], r32s[:, 4 * rc : 4 * rc + 1, :]
                )
            for rc in range(C):
                dmae = nc.sync if rc % 2 == 0 else nc.scalar
                dmae.dma_start(
                    out=gath[:, 4 * rc : 4 * rc + 1, :],
                    in_=cc_out_a[rc * P : (rc + 1) * P, 0:D].rearrange(
                        "(kc p) d -> p kc d", p=P
                    ),
                )
            for rc in range(C):
                dmae2 = nc.scalar if rc % 2 == 0 else nc.sync
                dmae2.dma_start(
                    out=r32s[:, 4 * rc + 1 : 4 * rc + 4, :],
                    in_=cc_out_b[rc * 3 * P : (rc + 1) * 3 * P, D : D + 1].rearrange(
                        "(kc p) d -> p kc d", p=P
                    ),
                )
                nc.scalar.copy(
                    r32f[:, 4 * rc + 1 : 4 * rc + 4, :],
                    r32s[:, 4 * rc + 1 : 4 * rc + 4, :],
                )
            for rc in range(C):
                dmae = nc.sync if rc % 2 == 0 else nc.scalar
                dmae.dma_start(
                    out=gath[:, 4 * rc + 1 : 4 * rc + 4, :],
                    in_=cc_out_b[rc * 3 * P : (rc + 1) * 3 * P, 0:D].rearrange(
                        "(kc p) d -> p kc d", p=P
                    ),
                )

            # lhsT: all rows x own 256 cols, scaled to 32/||row||, fp8
            lhsT_sb = big_pool.tile([P, KC, DC], FP8)
            for kc in range(KC):
                nc.vector.tensor_scalar_mul(
                    lhsT_sb[:, kc, :], xc_sb[:, kc, :], r32f[:, kc, :]
                )

            # ---------------- main Gram matmul (K-outer, DoubleRow) -------
            # 8 held PSUM tiles: (dsub, j) = own-128-col-half x 512-block.
            ps_big = [
                mm_psum.tile([P, D], F32, tag=f"big{d}", name=f"ps_big{d}")
                for d in range(2)
            ]
            # DoubleRow pairs of kc planes, ordered by data arrival:
            # chunk-a pairs (stride-4) first, then chunk-b pairs.
            kc_pairs = [(0, 4), (8, 4), (16, 4), (24, 4)]
            for g in range(4):
                kc_pairs += [(8 * g + 1, 1), (8 * g + 5, 1), (8 * g + 3, 4)]
            for step, (lo, st) in enumerate(kc_pairs):
                for n in range(8):
                    dsub, j = n // 4, n % 4
                    nc.tensor.matmul(
                        ps_big[dsub][:, ts(j, 512)],
                        lhsT=lhsT_sb[:, lo : lo + st + 1 : st, ts(dsub, P)],
                        rhs=gath[:, lo : lo + st + 1 : st, ts(j, 512)],
                        start=(step == 0),
                        stop=(step == len(kc_pairs) - 1),
                        perf_mode=DR,
                    )

            # ---------------- S epilogue (hidden under the matmuls) -------
            s_sum = const_pool.tile([NCLS, D + 1], F32)
            nc.gpsimd.dma_start(out=s_sum[:], in_=cc_s_out[:, :])
            s_scr = const_pool.tile([NCLS, D], BF16)
            s_acc = acc_pool.tile([NCLS, 1], F32)
            nc.scalar.activation(
                s_scr[:], s_sum[:, 0:D], AF.Square, accum_out=s_acc[:]
            )
            n2_scr = const_pool.tile([NCLS, 1], BF16)
            n2_acc = acc_pool.tile([NCLS, 1], F32)
            nc.scalar.activation(
                n2_scr[:], s_sum[:, D : D + 1], AF.Square, accum_out=n2_acc[:]
            )

            # ---------------- epilogue: square-accumulate -> F1 -----------
            # big0 on ACT; big1 on DVE (PSUM allows one input per DVE op,
            # so copy out first) — the two tails run in parallel.
            scr = ep_pool.tile([P, D], BF16, tag="scr0")
            red = red_pool.tile([P, 1], F32, tag="red0")
            nc.scalar.activation(scr[:], ps_big[0][:], AF.Square, accum_out=red[:])
            nc.vector.tensor_tensor(acc_f1[:], acc_f1[:], red[:], ALU.add)

            cpy = ep_pool.tile([P, D], BF16, tag="scr1")
            nc.vector.tensor_copy(cpy[:], ps_big[1][:])
            sq2 = ep_pool.tile([P, D], BF16, tag="scr2")
            nc.vector.tensor_tensor(sq2[:], cpy[:], cpy[:], ALU.mult)
            red2 = red_pool.tile([P, 1], F32, tag="red1")
            nc.vector.tensor_reduce(red2[:], sq2[:], mybir.AxisListType.X, ALU.add)
            nc.vector.tensor_tensor(acc_f1[:], acc_f1[:], red2[:], ALU.add)

            # ---------------- write partials ----------------
            out_sb = const_pool.tile([P, 4], F32)
            nc.vector.memset(out_sb[:], 0.0)
            nc.scalar.copy(out_sb[:, 0:1], acc_f1[:])
            nc.scalar.copy(out_sb[0:NCLS, 1:2], s_acc[:])
            nc.scalar.copy(out_sb[0:NCLS, 2:3], n2_acc[:])
            nc.sync.dma_start(out=out_parts[:, :], in_=out_sb[:])

    _split_multi_waits(nc)
    return nc


def make_in_maps(ys, labels, B, D, C, S=4):
    """Shard host inputs into per-core input maps."""
    import ml_dtypes

    ys = np.ascontiguousarray(ys, dtype=np.float32)
    labels = np.asarray(labels).astype(np.int64)
    Bs = B // C
    RT = Bs // P
    DC = D // C
    ys_f8 = ys.astype(ml_dtypes.float8_e4m3)
    in_maps = []
    for k in range(C):
        lab_k = labels[k * Bs : (k + 1) * Bs].reshape(RT, P)
        oh = np.zeros((P, RT * NCLS), dtype=np.float32)
        for t in range(RT):
            oh[np.arange(P), t * NCLS + lab_k[t]] = 1.0
        in_maps.append(
            {
                "ys_mine": ys[k * Bs : (k + 1) * Bs],
                "xcols": np.ascontiguousarray(
                    ys_f8[:, k * DC : (k + 1) * DC]
                    .reshape(B // P, P, DC)
                    .transpose(1, 0, 2)
                    .reshape(P, (B // P) * DC)
                ),
                "oh_mine": oh,
            }
        )
    return in_maps


def combine_parts(parts_list, B):
    """parts_list: per-core [128, 4] f32 partials -> scalar loss."""
    f1_acc = 0.0
    for p in parts_list:
        p = np.asarray(p, dtype=np.float64)
        f1_acc += p[:, 0].sum()
    p0 = np.asarray(parts_list[0], dtype=np.float64)
    f1 = f1_acc / SCALE**4
    ss = p0[0:NCLS, 1].sum() / SCALE**2
    n2 = p0[0:NCLS, 2].sum()
    total = (f1 - B) / 2.0 + 2.0 * (n2 - ss)
    n_pair = B * (B - 1) // 2
    return np.float32(total / n_pair)


_CACHED = {}


def kernel(ys: np.ndarray, labels: np.ndarray) -> np.ndarray:
    B, D = ys.shape
    C = 8
    key = (B, D, C)
    if key not in _CACHED:
        _CACHED[key] = build_gram_loss(B=B, D=D, C=C)
    nc = _CACHED[key]

    from concourse.bass_utils import run_bass_kernel_spmd

    in_maps = make_in_maps(np.asarray(ys), np.asarray(labels), B, D, C)
    res = run_bass_kernel_spmd(nc, in_maps, core_ids=list(range(C)))
    parts = [res.results[i]["out_parts"] for i in range(C)]
    return combine_parts(parts, B)


if __name__ == "__main__":
    nc = build_gram_loss()
    print("built ok:", len(nc.m.functions[0].blocks), "blocks")



# revision 11
# speedup vs baseline: 5.5870x; 5.5870x over previous
"""Contrastive-loss kernel for Trainium2, 8 NeuronCores — r2-scaled feature-Gram.

Math
----
reference:
    yn  = ys / clip(||ys||, 1e-6)          (row-normalize)
    cos = yn @ yn.T                         [B, B]
    pair_loss = same ? relu(2 - cos)^2 : cos^2
    loss = sum(strict_lower(pair_loss)) / (B*(B-1)/2)

Since margin M = 2 and cos <= 1: relu(2-cos)^2 = (2-cos)^2 = cos^2 + 4*(1-cos),
so summing over the strict lower triangle (diagonal terms: cos_ii = 1):
    sum_{i>j} pair_loss = (F1 - B)/2 + 2*(N2 - SS)
with
    F1 = sum_ij cos_ij^2 = ||Yn^T Yn||_F^2    (feature Gram, D x D)
    N2 = sum_c n_c^2 = ||O^T O||_F^2          (O = one-hot labels [B, 10])
    SS = sum_c ||S_c||^2 = ||O^T Yn||_F^2     (S_c = sum of yn rows, label c)

Key identities that kill the collectives (the entire baseline bottleneck —
its three DRAM collectives cost 272us in the cost model):

  * M = Yn^T Yn = X^T R^2 X  with R = diag(1/||x_r||): only the LHS of the
    matmul needs the row scaling — the rhs is the RAW fp8 matrix straight
    from the host.  Per-core lhsT = own 256 feature columns (host rotates
    columns so each core's slice sits at 0:256) scaled by r32^2.
  * S-slice: S[:, own 256 features] = (R O)^T X[:, own] — lhsT = r-scaled
    one-hot, rhs = raw own-feature columns.  sum over cores of
    ||S_slice||^2 = SS exactly (features partition the columns).
  * N2 = ||O^T O||_F^2 computed redundantly on every core from the raw
    one-hot (exact integers).

The ONLY cross-core data needed is r32 = 32/||x_r|| for all 4096 rows:
one AllGather of [128, 4] f32 per core -> [1024, 4] (16 KB), ~15.4us.
Everything else (8.4 MB raw fp8 rows) comes from the host via DMA, which
runs on otherwise-idle queues concurrently with the norm/gather phase.

Device schedule (SPMD, 8 cores):
  SP   : ys tiles 0,2 -> xplus pair-chunks (16 x 1.6us) -> out DMAs
  Pool : ys tiles 1,3, one-hot, memsets, collective
  ACT  : act-table preload, Square(t0), Square(t3), sqrt, cc_in DMA,
         lhsT odd planes, aux/H1/H2 epilogues
  DVE  : TTR(t1), TTR(t2), eps-clip/recip/x32, gather readback, r32^2,
         lhsT even planes + RO planes, final adds
  PE   : optional warm-up matmuls (keeps the p-state ramp hot during the
         collective), then H1 = dsub0 Gram + S + OTO interleaved (PSUM
         banks 0-3 + one aux bank), then H2 = dsub1 (banks 4-7 after the
         tiny aux epilogue frees them).
"""

import os
import sys

for _p in ("/opt/trn_rl_repo", "/root/.axon_site/_ro/trn_rl_repo"):
    if _p not in sys.path and os.path.isdir(_p):
        sys.path.append(_p)

import numpy as np

import concourse.bass as bass
import concourse.mybir as mybir
import concourse.tile as tile
from concourse.bass import ds, ts  # noqa: F401

F32 = mybir.dt.float32
BF16 = mybir.dt.bfloat16
FP8 = mybir.dt.float8e4
AF = mybir.ActivationFunctionType
ALU = mybir.AluOpType
DR = mybir.MatmulPerfMode.DoubleRow

P = 128       # partitions
NCLS = 10     # label classes
NCW = 16      # one-hot tile width (padded: 10-wide fp8 DR ldweights fails walrus)
SCALE = 32.0  # fp8 pre-scale on r32 = SCALE/||row||
NJUNK = 0     # PE warm-up matmuls during the collective (tuned empirically)


def _split_multi_waits(nc):
    """Split instructions carrying >1 semaphore wait.

    The walrus in this environment rejects compute instructions with more
    than one sync-wait command ("Too many sync wait commands"). Move the
    extra waits onto standalone EventSemaphore instructions inserted just
    before, on the same engine — semantically identical (the engine's
    sequencer blocks on each in order).
    """
    n_split = 0
    for fn in nc.m.functions:
        for bb in fn.blocks:
            new_insts = []
            for ins in bb.instructions:
                si = ins.sync_info
                if (
                    si is not None
                    and len(si.on_wait) > 1
                    and not isinstance(ins, mybir.InstEventSemaphore)
                ):
                    extra = list(si.on_wait[1:])
                    ins.sync_info = mybir.SyncInfo(
                        on_wait=[si.on_wait[0]], on_update=list(si.on_update)
                    )
                    for w in extra:
                        n_split += 1
                        ev = mybir.InstEventSemaphore(
                            name=f"antsplitwait_{n_split}_{ins.name}",
                            engine=ins.engine,
                            ins=[],
                            outs=[],
                            sync_info=mybir.SyncInfo(on_wait=[w], on_update=[]),
                            bass_nofuse=True,
                        )
                        new_insts.append(ev)
                new_insts.append(ins)
            bb.instructions = new_insts
    return n_split


def build_gram_loss(B=4096, D=2048, C=8, S=4):
    """Build the SPMD bass program (one nc, run on C cores). S unused."""
    assert B == 4096 and D == 2048 and C == 8
    Bs = B // C          # 512 rows per core
    RT = Bs // P         # 4 own row-tiles
    KC = B // P          # 32 row planes of 128
    NPAIR = KC // 2      # 16 DoubleRow plane pairs
    DC = D // C          # 256 own feature columns
    JW = 512             # matmul j-block width (one PSUM bank)

    nc = bass.Bass(num_devices=C)

    ys_mine = nc.dram_tensor("ys_mine", [Bs, D], BF16, kind="ExternalInput")
    xplus = nc.dram_tensor("xplus", [P, KC * D], FP8, kind="ExternalInput")
    oh_all = nc.dram_tensor("oh_all", [P, KC * NCW], FP8, kind="ExternalInput")
    out_parts = nc.dram_tensor("out_parts", [P, 4], F32, kind="ExternalOutput")

    cc_in = nc.dram_tensor("cc_in", [P, RT], F32)
    cc_out = nc.dram_tensor("cc_out", [C * P, RT], F32, addr_space="Shared")

    with tile.TileContext(nc) as tc:
        with (
            tc.tile_pool(name="const", bufs=1) as const_pool,
            tc.tile_pool(name="big", bufs=1) as big_pool,
            tc.tile_pool(name="ysin", bufs=1) as ys_pool,
            tc.tile_pool(name="sqscr", bufs=1) as sq_pool,
            tc.tile_pool(name="small", bufs=4) as small_pool,
            tc.tile_pool(name="ep", bufs=1) as ep_pool,
            tc.tile_pool(name="mm", bufs=1, space="PSUM") as mm_psum,
        ):
            # ---------------- tiles ----------------
            xp_sb = big_pool.tile([P, KC, D], FP8)
            lhsT_sb = big_pool.tile([P, KC, DC], FP8)
            oh_sb = const_pool.tile([P, KC, NCW], FP8)
            ro_sb = const_pool.tile([P, KC, NCW], FP8)
            r32g = const_pool.tile([P, KC], F32)
            r32sq = const_pool.tile([P, KC], F32)

            ys_t = [ys_pool.tile([P, D], BF16, name=f"ys_t{t}") for t in range(RT)]
            sq_a = sq_pool.tile([P, D], BF16)   # ACT square scratch
            sq_d = sq_pool.tile([P, D], BF16)   # DVE square scratch
            sq_d2 = sq_pool.tile([P, D], BF16)  # DVE accum-sum scratch
            ssq = small_pool.tile([P, RT], F32)
            norm_t = small_pool.tile([P, RT], F32)
            rcp_t = small_pool.tile([P, RT], F32)
            r32_own = small_pool.tile([P, RT], F32)
            dummy = small_pool.tile([P, 1], F32)
            dummy_o = small_pool.tile([P, 1], BF16)
            out_sb = const_pool.tile([P, 4], F32)

            # ---------------- t=0: DMAs + ACT table preload ----------------
            # SP: ys tiles 0,2 then the 16 xplus pair-chunks.
            nc.sync.dma_start(out=ys_t[0][:], in_=ys_mine[ts(0, P), :])
            nc.sync.dma_start(out=ys_t[2][:], in_=ys_mine[ts(2, P), :])
            for g in range(NPAIR):
                nc.sync.dma_start(
                    out=xp_sb[:, 2 * g : 2 * g + 2, :],
                    in_=xplus[:, 2 * g * D : (2 * g + 2) * D].rearrange(
                        "p (k d) -> p k d", k=2
                    ),
                )
            # Pool: ys tiles 1,3 + one-hot + memsets.
            nc.gpsimd.memset(dummy[:], 1.0)
            nc.gpsimd.dma_start(out=ys_t[1][:], in_=ys_mine[ts(1, P), :])
            nc.gpsimd.dma_start(out=ys_t[3][:], in_=ys_mine[ts(3, P), :])
            nc.gpsimd.dma_start(out=oh_sb[:, :, :], in_=oh_all[:, :].rearrange(
                "p (k c) -> p k c", c=NCW
            ))
            nc.vector.memset(out_sb[:], 0.0)

            # ACT: pay the Square activation-table load before data arrives.
            nc.scalar.activation(dummy_o[:], dummy[:], AF.Square)

            # ---------------- norms of own 512 rows ----------------
            # ACT squares tiles 0,3; DVE squares tiles 1,2 (TTR mult+add).
            for t, eng in ((0, "act"), (1, "dve"), (3, "act"), (2, "dve")):
                if eng == "act":
                    nc.scalar.activation(
                        sq_a[:], ys_t[t][:], AF.Square,
                        accum_out=ssq[:, t : t + 1],
                    )
                else:
                    nc.vector.tensor_tensor(
                        sq_d[:], ys_t[t][:], ys_t[t][:], ALU.mult
                    )
                    nc.vector.tensor_scalar(
                        sq_d2[:], sq_d[:], 1.0, None, ALU.mult, ALU.add,
                        accum_out=ssq[:, t : t + 1],
                    )
            nc.scalar.sqrt(norm_t[:], ssq[:])
            nc.vector.tensor_scalar_max(norm_t[:], norm_t[:], 1e-6)
            nc.vector.reciprocal(rcp_t[:], norm_t[:])
            nc.vector.tensor_scalar_mul(r32_own[:], rcp_t[:], SCALE)

            # ---------------- the one collective: AllGather r32 ----------
            nc.scalar.dma_start(out=cc_in[:, :], in_=r32_own[:])
            nc.gpsimd.collective_compute(
                "AllGather",
                ALU.bypass,
                replica_groups=[list(range(C))],
                ins=[cc_in[:, :]],
                outs=[cc_out[:, :]],
            )
            # readback on ACT (idle during the collective; DVE cannot issue
            # DMAs). cc_out[(r p), t] -> [p, r, t]: column index r*4 + t =
            # global plane kc.
            nc.scalar.dma_start(
                out=r32g[:, :].rearrange("p (r t) -> p r t", t=RT),
                in_=cc_out[:, :].rearrange("(r p) t -> p r t", p=P),
            )
            nc.vector.tensor_tensor(r32sq[:], r32g[:], r32g[:], ALU.mult)

            # ---------------- lhsT + RO builds (pipelined under PE) -------
            # DVE: even planes + both RO planes per pair; ACT: odd planes.
            for g in range(NPAIR):
                k0, k1 = 2 * g, 2 * g + 1
                nc.vector.tensor_scalar_mul(
                    lhsT_sb[:, k0, :], xp_sb[:, k0, 0:DC], r32sq[:, k0 : k0 + 1]
                )
                nc.vector.tensor_scalar_mul(
                    ro_sb[:, k0, :], oh_sb[:, k0, :], r32g[:, k0 : k0 + 1]
                )
                nc.vector.tensor_scalar_mul(
                    ro_sb[:, k1, :], oh_sb[:, k1, :], r32g[:, k1 : k1 + 1]
                )
                nc.scalar.mul(
                    lhsT_sb[:, k1, :], xp_sb[:, k1, 0:DC], r32sq[:, k1 : k1 + 1]
                )

            # ---------------- PE ----------------
            # warm-up: keep the PE p-state ramp hot during the collective.
            if NJUNK:
                ps_junk = mm_psum.tile([P, D], F32, tag="big1", name="ps_junk")
                for _ in range(NJUNK):
                    nc.tensor.matmul(
                        ps_junk[:, 1536:2048],
                        lhsT=xp_sb[:, 0:2, 0:P],
                        rhs=xp_sb[:, 0:2, 1536:2048],
                        start=True,
                        stop=True,
                        perf_mode=DR,
                    )

            ps_aux = mm_psum.tile([P, D], F32, tag="big1", name="ps_aux")
            ps_h1 = mm_psum.tile([P, D], F32, tag="big0", name="ps_h1")
            ps_h2 = mm_psum.tile([P, D], F32, tag="big1", name="ps_h2")

            # H1: dsub0 Gram into banks 0-3; S + O^T O into one aux bank.
            for g in range(NPAIR):
                pr = slice(2 * g, 2 * g + 2)
                st, sp = g == 0, g == NPAIR - 1
                for j in range(D // JW):
                    nc.tensor.matmul(
                        ps_h1[:, ts(j, JW)],
                        lhsT=lhsT_sb[:, pr, 0:P],
                        rhs=xp_sb[:, pr, ts(j, JW)],
                        start=st,
                        stop=sp,
                        perf_mode=DR,
                    )
                nc.tensor.matmul(
                    ps_aux[0:NCW, 0:DC],
                    lhsT=ro_sb[:, pr, :],
                    rhs=xp_sb[:, pr, 0:DC],
                    start=st,
                    stop=sp,
                    perf_mode=DR,
                )
                nc.tensor.matmul(
                    ps_aux[0:NCW, 512 : 512 + NCW],
                    lhsT=oh_sb[:, pr, :],
                    rhs=oh_sb[:, pr, :],
                    start=st,
                    stop=sp,
                    perf_mode=DR,
                )

            # aux epilogue (ACT, frees banks 4-7 for H2):
            #   SS partial: sum over own-feature slice of S^2  (x SCALE^2)
            #   N2:         sum over (O^T O)^2 — exact integers
            s_scr = ep_pool.tile([NCW, DC], BF16)
            s_acc = small_pool.tile([NCW, 1], F32)
            nc.scalar.activation(
                s_scr[:], ps_aux[0:NCW, 0:DC], AF.Square, accum_out=s_acc[:]
            )
            o_scr = ep_pool.tile([NCW, NCW], BF16)
            n2_acc = small_pool.tile([NCW, 1], F32)
            nc.scalar.activation(
                o_scr[:], ps_aux[0:NCW, 512 : 512 + NCW], AF.Square,
                accum_out=n2_acc[:],
            )

            # H2: dsub1 Gram into banks 4-7.
            for g in range(NPAIR):
                pr = slice(2 * g, 2 * g + 2)
                st, sp = g == 0, g == NPAIR - 1
                for j in range(D // JW):
                    nc.tensor.matmul(
                        ps_h2[:, ts(j, JW)],
                        lhsT=lhsT_sb[:, pr, P:DC],
                        rhs=xp_sb[:, pr, ts(j, JW)],
                        start=st,
                        stop=sp,
                        perf_mode=DR,
                    )

            # H1 epilogue runs on ACT during H2; H2 epilogue at the tail.
            h1_scr = ep_pool.tile([P, D], BF16)
            red1 = small_pool.tile([P, 1], F32)
            nc.scalar.activation(h1_scr[:], ps_h1[:], AF.Square, accum_out=red1[:])

            # first output DMA: F1(dsub0), SS partial, N2 — all ready mid-H2
            nc.scalar.copy(out_sb[:, 0:1], red1[:])
            nc.scalar.copy(out_sb[0:NCLS, 1:2], s_acc[0:NCLS, :])
            nc.scalar.copy(out_sb[0:NCLS, 2:3], n2_acc[0:NCLS, :])
            nc.sync.dma_start(out=out_parts[:, 0:3], in_=out_sb[:, 0:3])

            h2_scr = ep_pool.tile([P, D], BF16)
            red2 = small_pool.tile([P, 1], F32)
            nc.scalar.activation(h2_scr[:], ps_h2[:], AF.Square, accum_out=red2[:])
            nc.scalar.copy(out_sb[:, 3:4], red2[:])
            nc.sync.dma_start(out=out_parts[:, 3:4], in_=out_sb[:, 3:4])

    _split_multi_waits(nc)
    return nc


def make_in_maps(ys, labels, B, D, C, S=4):
    """Shard host inputs into per-core input maps (dtype packing only)."""
    import ml_dtypes

    ys = np.ascontiguousarray(ys, dtype=np.float32)
    labels = np.asarray(labels).astype(np.int64)
    Bs = B // C
    KC = B // P
    DC = D // C
    ys_f8 = ys.astype(ml_dtypes.float8_e4m3)
    ys_bf = ys.astype(ml_dtypes.bfloat16)

    # one-hot in plane layout [p, kc*10 + c] — identical on every core
    lab2 = labels.reshape(KC, P)
    oh = np.zeros((P, KC, NCW), dtype=ml_dtypes.float8_e4m3)
    oh[np.arange(P)[:, None], np.arange(KC)[None, :], lab2.T] = 1.0
    oh = np.ascontiguousarray(oh.reshape(P, KC * NCW))

    in_maps = []
    for k in range(C):
        # rotate columns so core k's 256 features sit at 0:DC, then pack
        # rows into [p, kc*D + d] with global row = kc*128 + p
        rot = np.concatenate(
            [ys_f8[:, k * DC :], ys_f8[:, : k * DC]], axis=1
        )
        xp = np.ascontiguousarray(
            rot.reshape(KC, P, D).transpose(1, 0, 2).reshape(P, KC * D)
        )
        in_maps.append(
            {
                "ys_mine": np.ascontiguousarray(ys_bf[k * Bs : (k + 1) * Bs]),
                "xplus": xp,
                "oh_all": oh,
            }
        )
    return in_maps


def combine_parts(parts_list, B):
    """parts_list: per-core [128, 4] f32 partials -> scalar loss."""
    f1 = 0.0
    ss = 0.0
    for p in parts_list:
        p = np.asarray(p, dtype=np.float64)
        f1 += p[:, 0].sum() + p[:, 3].sum()
        ss += p[0:NCLS, 1].sum()
    f1 /= SCALE**4
    ss /= SCALE**2
    n2 = np.asarray(parts_list[0], dtype=np.float64)[0:NCLS, 2].sum()
    total = (f1 - B) / 2.0 + 2.0 * (n2 - ss)
    n_pair = B * (B - 1) // 2
    return np.float32(total / n_pair)


_CACHED = {}


def kernel(ys: np.ndarray, labels: np.ndarray) -> np.ndarray:
    B, D = ys.shape
    C = 8
    key = (B, D, C)
    if key not in _CACHED:
        _CACHED[key] = build_gram_loss(B=B, D=D, C=C)
    nc = _CACHED[key]

    from concourse.bass_utils import run_bass_kernel_spmd

    in_maps = make_in_maps(np.asarray(ys), np.asarray(labels), B, D, C)
    res = run_bass_kernel_spmd(nc, in_maps, core_ids=list(range(C)))
    parts = [res.results[i]["out_parts"] for i in range(C)]
    return combine_parts(parts, B)


if __name__ == "__main__":
    nc = build_gram_loss()
    print("built ok:", len(nc.m.functions[0].blocks), "blocks")


# revision 18
# speedup vs baseline: 5.8601x; 1.0489x over previous
"""Contrastive-loss kernel for Trainium2, 8 NeuronCores — r2-scaled feature-Gram.

Math
----
reference:
    yn  = ys / clip(||ys||, 1e-6)          (row-normalize)
    cos = yn @ yn.T                         [B, B]
    pair_loss = same ? relu(2 - cos)^2 : cos^2
    loss = sum(strict_lower(pair_loss)) / (B*(B-1)/2)

Since margin M = 2 and cos <= 1: relu(2-cos)^2 = (2-cos)^2 = cos^2 + 4*(1-cos),
so summing over the strict lower triangle (diagonal terms: cos_ii = 1):
    sum_{i>j} pair_loss = (F1 - B)/2 + 2*(N2 - SS)
with
    F1 = sum_ij cos_ij^2 = ||Yn^T Yn||_F^2    (feature Gram, D x D)
    N2 = sum_c n_c^2 = ||O^T O||_F^2          (O = one-hot labels [B, 10])
    SS = sum_c ||S_c||^2 = ||O^T Yn||_F^2     (S_c = sum of yn rows, label c)

Key identities that kill the collectives (the entire baseline bottleneck —
its three DRAM collectives cost 272us in the cost model):

  * M = Yn^T Yn = X^T R^2 X  with R = diag(1/||x_r||): only the LHS of the
    matmul needs the row scaling — the rhs is the RAW fp8 matrix straight
    from the host.  Per-core lhsT = own 256 feature columns (host rotates
    columns so each core's slice sits at 0:256) scaled by r32^2.
  * S-slice: S[:, own 256 features] = (R O)^T X[:, own] — lhsT = r-scaled
    one-hot, rhs = raw own-feature columns.  sum over cores of
    ||S_slice||^2 = SS exactly (features partition the columns).
  * N2 = ||O^T O||_F^2 computed redundantly on every core from the raw
    one-hot (exact integers).

The ONLY cross-core data needed is r32 = 32/||x_r|| for all 4096 rows:
one AllGather of [128, 4] f32 per core -> [1024, 4] (16 KB), ~15.4us.
Everything else (8.4 MB raw fp8 rows) comes from the host via DMA, which
runs on otherwise-idle queues concurrently with the norm/gather phase.

Device schedule (SPMD, 8 cores):
  SP   : ys tiles 0,2 -> xplus pair-chunks (16 x 1.6us) -> out DMAs
  Pool : ys tiles 1,3, one-hot, memsets, collective
  ACT  : act-table preload, Square(t0), Square(t3), sqrt, cc_in DMA,
         lhsT odd planes, aux/H1/H2 epilogues
  DVE  : TTR(t1), TTR(t2), eps-clip/recip/x32, gather readback, r32^2,
         lhsT even planes + RO planes, final adds
  PE   : optional warm-up matmuls (keeps the p-state ramp hot during the
         collective), then H1 = dsub0 Gram + S + OTO interleaved (PSUM
         banks 0-3 + one aux bank), then H2 = dsub1 (banks 4-7 after the
         tiny aux epilogue frees them).
"""

import os
import sys

for _p in ("/opt/trn_rl_repo", "/root/.axon_site/_ro/trn_rl_repo"):
    if _p not in sys.path and os.path.isdir(_p):
        sys.path.append(_p)

import numpy as np

import concourse.bass as bass
import concourse.mybir as mybir
import concourse.tile as tile
from concourse.bass import ds, ts  # noqa: F401

F32 = mybir.dt.float32
BF16 = mybir.dt.bfloat16
FP8 = mybir.dt.float8e4
AF = mybir.ActivationFunctionType
ALU = mybir.AluOpType
DR = mybir.MatmulPerfMode.DoubleRow

P = 128       # partitions
NCLS = 10     # label classes
NCW = 16      # one-hot tile width (padded: 10-wide fp8 DR ldweights fails walrus)
SCALE = 32.0  # fp8 pre-scale on r32 = SCALE/||row||
NJUNK = 0     # PE warm-up matmuls during the collective (tuned empirically)


def _split_multi_waits(nc):
    """Split instructions carrying >1 semaphore wait.

    The walrus in this environment rejects compute instructions with more
    than one sync-wait command ("Too many sync wait commands"). Move the
    extra waits onto standalone EventSemaphore instructions inserted just
    before, on the same engine — semantically identical (the engine's
    sequencer blocks on each in order).
    """
    n_split = 0
    for fn in nc.m.functions:
        for bb in fn.blocks:
            new_insts = []
            for ins in bb.instructions:
                si = ins.sync_info
                if (
                    si is not None
                    and len(si.on_wait) > 1
                    and not isinstance(ins, mybir.InstEventSemaphore)
                ):
                    extra = list(si.on_wait[1:])
                    ins.sync_info = mybir.SyncInfo(
                        on_wait=[si.on_wait[0]], on_update=list(si.on_update)
                    )
                    for w in extra:
                        n_split += 1
                        ev = mybir.InstEventSemaphore(
                            name=f"antsplitwait_{n_split}_{ins.name}",
                            engine=ins.engine,
                            ins=[],
                            outs=[],
                            sync_info=mybir.SyncInfo(on_wait=[w], on_update=[]),
                            bass_nofuse=True,
                        )
                        new_insts.append(ev)
                new_insts.append(ins)
            bb.instructions = new_insts
    return n_split


def build_gram_loss(B=4096, D=2048, C=8, S=4):
    """Build the SPMD bass program (one nc, run on C cores). S unused."""
    assert B == 4096 and D == 2048 and C == 8
    Bs = B // C          # 512 rows per core
    RT = Bs // P         # 4 own row-tiles
    KC = B // P          # 32 row planes of 128
    NPAIR = KC // 2      # 16 DoubleRow plane pairs
    DC = D // C          # 256 own feature columns
    JW = 512             # matmul j-block width (one PSUM bank)

    nc = bass.Bass(num_devices=C)

    ys_mine = nc.dram_tensor("ys_mine", [Bs, D], BF16, kind="ExternalInput")
    xplus = nc.dram_tensor("xplus", [P, KC * D], FP8, kind="ExternalInput")
    oh_all = nc.dram_tensor("oh_all", [P, KC * NCW], FP8, kind="ExternalInput")
    out_parts = nc.dram_tensor("out_parts", [P, 4], F32, kind="ExternalOutput")

    cc_in = nc.dram_tensor("cc_in", [P, RT], F32)
    cc_out = nc.dram_tensor("cc_out", [C * P, RT], F32, addr_space="Shared")

    with tile.TileContext(nc) as tc:
        with (
            tc.tile_pool(name="const", bufs=1) as const_pool,
            tc.tile_pool(name="big", bufs=1) as big_pool,
            tc.tile_pool(name="ysin", bufs=1) as ys_pool,
            tc.tile_pool(name="sqscr", bufs=1) as sq_pool,
            tc.tile_pool(name="small", bufs=4) as small_pool,
            tc.tile_pool(name="ep", bufs=1) as ep_pool,
            tc.tile_pool(name="mm", bufs=1, space="PSUM") as mm_psum,
        ):
            # ---------------- tiles ----------------
            xp_sb = big_pool.tile([P, KC, D], FP8)
            lhsT_sb = big_pool.tile([P, KC, DC], FP8)
            oh_sb = const_pool.tile([P, KC, NCW], FP8)
            ro_sb = const_pool.tile([P, KC, NCW], FP8)
            r32g = const_pool.tile([P, KC], F32)
            r32sq = const_pool.tile([P, KC], F32)

            ys_t = [ys_pool.tile([P, D], BF16, name=f"ys_t{t}") for t in range(RT)]
            sq_a = sq_pool.tile([P, D], BF16)   # ACT square scratch
            sq_d = sq_pool.tile([P, D], BF16)   # DVE square scratch
            sq_d2 = sq_pool.tile([P, D], BF16)  # DVE accum-sum scratch
            ssq = small_pool.tile([P, RT], F32)
            norm_t = small_pool.tile([P, RT], F32)
            rcp_t = small_pool.tile([P, RT], F32)
            r32_own = small_pool.tile([P, RT], F32)
            dummy = small_pool.tile([P, 1], F32)
            dummy_o = small_pool.tile([P, 1], BF16)
            out_sb = const_pool.tile([P, 4], F32)

            # ---------------- t=0: DMAs + ACT table preload ----------------
            # SP: ys tiles 0,2 then the 16 xplus pair-chunks.
            nc.sync.dma_start(out=ys_t[0][:], in_=ys_mine[ts(0, P), :])
            nc.sync.dma_start(out=ys_t[2][:], in_=ys_mine[ts(2, P), :])
            for g in range(NPAIR):
                nc.sync.dma_start(
                    out=xp_sb[:, 2 * g : 2 * g + 2, :],
                    in_=xplus[:, 2 * g * D : (2 * g + 2) * D].rearrange(
                        "p (k d) -> p k d", k=2
                    ),
                )
            # Pool: ys tiles 1,3 + one-hot + memsets.
            nc.gpsimd.memset(dummy[:], 1.0)
            nc.gpsimd.dma_start(out=ys_t[1][:], in_=ys_mine[ts(1, P), :])
            nc.gpsimd.dma_start(out=ys_t[3][:], in_=ys_mine[ts(3, P), :])
            nc.gpsimd.dma_start(out=oh_sb[:, :, :], in_=oh_all[:, :].rearrange(
                "p (k c) -> p k c", c=NCW
            ))
            nc.vector.memset(out_sb[:], 0.0)

            # ACT: pay the Square activation-table load before data arrives.
            nc.scalar.activation(dummy_o[:], dummy[:], AF.Square)

            # ---------------- norms of own 512 rows ----------------
            # ACT squares tiles 0,3; DVE squares tiles 1,2 (TTR mult+add).
            for t, eng in ((0, "act"), (1, "dve"), (3, "act"), (2, "dve")):
                if eng == "act":
                    nc.scalar.activation(
                        sq_a[:], ys_t[t][:], AF.Square,
                        accum_out=ssq[:, t : t + 1],
                    )
                else:
                    nc.vector.tensor_tensor(
                        sq_d[:], ys_t[t][:], ys_t[t][:], ALU.mult
                    )
                    nc.vector.tensor_scalar(
                        sq_d2[:], sq_d[:], 1.0, None, ALU.mult, ALU.add,
                        accum_out=ssq[:, t : t + 1],
                    )
            # O^T O chain needs only host data — run it pre-gather so its
            # epilogue is long done before H2 wants the aux banks, and the
            # early matmuls start the PE p-state ramp.
            ps_aux = mm_psum.tile([P, D], F32, tag="big1", name="ps_aux")
            for g in range(NPAIR):
                pr = slice(2 * g, 2 * g + 2)
                nc.tensor.matmul(
                    ps_aux[0:NCW, 512 : 512 + NCW],
                    lhsT=oh_sb[:, pr, :],
                    rhs=oh_sb[:, pr, :],
                    start=g == 0,
                    stop=g == NPAIR - 1,
                    perf_mode=DR,
                )
            # OTO epilogue on DVE (idle during the collective; ACT's in-order
            # queue must stay clear for sqrt -> cc_in -> readback).
            o_cpy = ep_pool.tile([NCW, NCW], BF16)
            o_sq = ep_pool.tile([NCW, NCW], BF16)
            o_s2 = ep_pool.tile([NCW, NCW], BF16)
            n2_acc = small_pool.tile([NCW, 1], F32)
            nc.vector.tensor_copy(o_cpy[:], ps_aux[0:NCW, 512 : 512 + NCW])
            nc.vector.tensor_tensor(o_sq[:], o_cpy[:], o_cpy[:], ALU.mult)
            nc.vector.tensor_scalar(
                o_s2[:], o_sq[:], 1.0, None, ALU.mult, ALU.add,
                accum_out=n2_acc[:],
            )

            nc.scalar.sqrt(norm_t[:], ssq[:])
            nc.vector.tensor_scalar_max(norm_t[:], norm_t[:], 1e-6)
            nc.vector.reciprocal(rcp_t[:], norm_t[:])
            nc.vector.tensor_scalar_mul(r32_own[:], rcp_t[:], SCALE)

            # ---------------- the one collective: AllGather r32 ----------
            nc.scalar.dma_start(out=cc_in[:, :], in_=r32_own[:])
            nc.gpsimd.collective_compute(
                "AllGather",
                ALU.bypass,
                replica_groups=[list(range(C))],
                ins=[cc_in[:, :]],
                outs=[cc_out[:, :]],
            )
            # readback on ACT (idle during the collective; DVE cannot issue
            # DMAs). cc_out[(r p), t] -> [p, r, t]: column index r*4 + t =
            # global plane kc.
            nc.scalar.dma_start(
                out=r32g[:, :].rearrange("p (r t) -> p r t", t=RT),
                in_=cc_out[:, :].rearrange("(r p) t -> p r t", p=P),
            )
            nc.vector.tensor_tensor(r32sq[:], r32g[:], r32g[:], ALU.mult)

            # ---------------- lhsT + RO builds (pipelined under PE) -------
            # DVE: even planes + both RO planes per pair; ACT: odd planes.
            for g in range(NPAIR):
                k0, k1 = 2 * g, 2 * g + 1
                nc.vector.tensor_scalar_mul(
                    lhsT_sb[:, k0, :], xp_sb[:, k0, 0:DC], r32sq[:, k0 : k0 + 1]
                )
                nc.vector.tensor_scalar_mul(
                    ro_sb[:, k0, :], oh_sb[:, k0, :], r32g[:, k0 : k0 + 1]
                )
                nc.vector.tensor_scalar_mul(
                    ro_sb[:, k1, :], oh_sb[:, k1, :], r32g[:, k1 : k1 + 1]
                )
                nc.scalar.mul(
                    lhsT_sb[:, k1, :], xp_sb[:, k1, 0:DC], r32sq[:, k1 : k1 + 1]
                )

            # ---------------- PE ----------------
            # warm-up: keep the PE p-state ramp hot during the collective.
            if NJUNK:
                ps_junk = mm_psum.tile([P, D], F32, tag="big1", name="ps_junk")
                for _ in range(NJUNK):
                    nc.tensor.matmul(
                        ps_junk[:, 1536:2048],
                        lhsT=xp_sb[:, 0:2, 0:P],
                        rhs=xp_sb[:, 0:2, 1536:2048],
                        start=True,
                        stop=True,
                        perf_mode=DR,
                    )

            ps_h1 = mm_psum.tile([P, D], F32, tag="big0", name="ps_h1")
            ps_h2 = mm_psum.tile([P, D], F32, tag="big1", name="ps_h2")

            # H1: dsub0 Gram into banks 0-3; S + O^T O into one aux bank.
            for g in range(NPAIR):
                pr = slice(2 * g, 2 * g + 2)
                st, sp = g == 0, g == NPAIR - 1
                for j in range(D // JW):
                    nc.tensor.matmul(
                        ps_h1[:, ts(j, JW)],
                        lhsT=lhsT_sb[:, pr, 0:P],
                        rhs=xp_sb[:, pr, ts(j, JW)],
                        start=st,
                        stop=sp,
                        perf_mode=DR,
                    )
                nc.tensor.matmul(
                    ps_aux[0:NCW, 0:DC],
                    lhsT=ro_sb[:, pr, :],
                    rhs=xp_sb[:, pr, 0:DC],
                    start=st,
                    stop=sp,
                    perf_mode=DR,
                )

            # aux epilogue (ACT, frees banks 4-7 for H2):
            #   SS partial: sum over own-feature slice of S^2  (x SCALE^2)
            #   N2:         sum over (O^T O)^2 — exact integers
            s_scr = ep_pool.tile([NCW, DC], BF16)
            s_acc = small_pool.tile([NCW, 1], F32)
            nc.scalar.activation(
                s_scr[:], ps_aux[0:NCW, 0:DC], AF.Square, accum_out=s_acc[:]
            )

            # H2: dsub1 Gram into banks 4-7.
            for g in range(NPAIR):
                pr = slice(2 * g, 2 * g + 2)
                st, sp = g == 0, g == NPAIR - 1
                for j in range(D // JW):
                    nc.tensor.matmul(
                        ps_h2[:, ts(j, JW)],
                        lhsT=lhsT_sb[:, pr, P:DC],
                        rhs=xp_sb[:, pr, ts(j, JW)],
                        start=st,
                        stop=sp,
                        perf_mode=DR,
                    )

            # H1 epilogue runs on DVE during H2 (keeping ACT free for the aux
            # epilogue, which gates H2's first matmul); H2 epilogue at tail.
            h1_cpy = ep_pool.tile([P, D], BF16)
            h1_sq = ep_pool.tile([P, D], BF16)
            h1_s2 = ep_pool.tile([P, D], BF16)
            red1 = small_pool.tile([P, 1], F32)
            nc.vector.tensor_copy(h1_cpy[:], ps_h1[:])
            nc.vector.tensor_tensor(h1_sq[:], h1_cpy[:], h1_cpy[:], ALU.mult)
            nc.vector.tensor_scalar(
                h1_s2[:], h1_sq[:], 1.0, None, ALU.mult, ALU.add,
                accum_out=red1[:],
            )

            # first output DMA: F1(dsub0), SS partial, N2 — all ready mid-H2
            nc.scalar.copy(out_sb[:, 0:1], red1[:])
            nc.scalar.copy(out_sb[0:NCLS, 1:2], s_acc[0:NCLS, :])
            nc.scalar.copy(out_sb[0:NCLS, 2:3], n2_acc[0:NCLS, :])
            nc.sync.dma_start(out=out_parts[:, 0:3], in_=out_sb[:, 0:3])

            h2_scr = ep_pool.tile([P, D], BF16)
            red2 = small_pool.tile([P, 1], F32)
            nc.scalar.activation(h2_scr[:], ps_h2[:], AF.Square, accum_out=red2[:])
            nc.sync.dma_start(out=out_parts[:, 3:4], in_=red2[:])

    _split_multi_waits(nc)
    return nc


def make_in_maps(ys, labels, B, D, C, S=4):
    """Shard host inputs into per-core input maps (dtype packing only)."""
    import ml_dtypes

    ys = np.ascontiguousarray(ys, dtype=np.float32)
    labels = np.asarray(labels).astype(np.int64)
    Bs = B // C
    KC = B // P
    DC = D // C
    ys_f8 = ys.astype(ml_dtypes.float8_e4m3)
    ys_bf = ys.astype(ml_dtypes.bfloat16)

    # one-hot in plane layout [p, kc*10 + c] — identical on every core
    lab2 = labels.reshape(KC, P)
    oh = np.zeros((P, KC, NCW), dtype=ml_dtypes.float8_e4m3)
    oh[np.arange(P)[:, None], np.arange(KC)[None, :], lab2.T] = 1.0
    oh = np.ascontiguousarray(oh.reshape(P, KC * NCW))

    in_maps = []
    for k in range(C):
        # rotate columns so core k's 256 features sit at 0:DC, then pack
        # rows into [p, kc*D + d] with global row = kc*128 + p
        rot = np.concatenate(
            [ys_f8[:, k * DC :], ys_f8[:, : k * DC]], axis=1
        )
        xp = np.ascontiguousarray(
            rot.reshape(KC, P, D).transpose(1, 0, 2).reshape(P, KC * D)
        )
        in_maps.append(
            {
                "ys_mine": np.ascontiguousarray(ys_bf[k * Bs : (k + 1) * Bs]),
                "xplus": xp,
                "oh_all": oh,
            }
        )
    return in_maps


def combine_parts(parts_list, B):
    """parts_list: per-core [128, 4] f32 partials -> scalar loss."""
    f1 = 0.0
    ss = 0.0
    for p in parts_list:
        p = np.asarray(p, dtype=np.float64)
        f1 += p[:, 0].sum() + p[:, 3].sum()
        ss += p[0:NCLS, 1].sum()
    f1 /= SCALE**4
    ss /= SCALE**2
    n2 = np.asarray(parts_list[0], dtype=np.float64)[0:NCLS, 2].sum()
    total = (f1 - B) / 2.0 + 2.0 * (n2 - ss)
    n_pair = B * (B - 1) // 2
    return np.float32(total / n_pair)


_CACHED = {}


def kernel(ys: np.ndarray, labels: np.ndarray) -> np.ndarray:
    B, D = ys.shape
    C = 8
    key = (B, D, C)
    if key not in _CACHED:
        _CACHED[key] = build_gram_loss(B=B, D=D, C=C)
    nc = _CACHED[key]

    from concourse.bass_utils import run_bass_kernel_spmd

    in_maps = make_in_maps(np.asarray(ys), np.asarray(labels), B, D, C)
    res = run_bass_kernel_spmd(nc, in_maps, core_ids=list(range(C)))
    parts = [res.results[i]["out_parts"] for i in range(C)]
    return combine_parts(parts, B)


if __name__ == "__main__":
    nc = build_gram_loss()
    print("built ok:", len(nc.m.functions[0].blocks), "blocks")
